# revision 16
# baseline (speedup 1.0000x reference)
"""Trainium2 Bass kernel for nn_EnhancedAutoformer (LearnableSeriesDecomp).

Computes, for x[B=64, L=2048, D=512]:
  - a per-sample kernel size k (tiny MLP on the temporal mean of x),
  - a per-sample softmax-normalized depthwise moving-average kernel of length
    k built from trend_weights[D, 50],
  - trend = depthwise conv (replicate padding), seasonal = x - trend.

Strategy (pure data parallelism over B across 8 NeuronCores; 8 samples/core):

The softmax weights factor as W[d, j] = E_j[d] / Z[d] with E = exp(tw).
trend_weights[:, :25] is initialized to the constant 1/25, so for taps
j < 25 the weight E_j is a per-sample *scalar*; only taps j >= 25 (2 of the
~27 used taps) vary across channels d. This turns the bulk of the depthwise
conv into a banded-Toeplitz matmul shared across all channels:

  trend[l, d] = invZ[d] * ( sum_{uniform j} E_j * x[clamp(l + d_j), d]
                          + sum_{resid r}  E_r[d] * x[clamp(l + d_r), d] )

On device, with output tiles [128 l-rows x 512 d] (l on partitions):
  - the uniform part is 2-3 TensorE matmuls per tile against small banded
    [128, 128] matrices (host-built, with replicate-pad clamping folded into
    the edge-tile matrices),
  - each residual tap r is a scaled copy xs_r = x * E_r[d] (VectorE) plus
    shifted-diagonal matmuls accumulated into the same PSUM tile,
  - epilogue: psum -> fp16 on ScalarE, trend = psc * invZ[d] and
    seasonal = x - trend on VectorE (all fp16 => 2x DVE mode), one fused
    [trend|seasonal] fp16 store per tile, upcast to fp32 on the host.

The whole matmul path runs in fp16 (PE lowers fp32 matmuls to two HW passes
and fp32 weights disable fast weight load): uniform weights are pre-rounded
to fp16 and that rounded value enters the normalizer Z, residual matrices
are 0/1, E-rows are pre-rounded before Z — so fp16 adds only per-element
rounding noise (~1e-3 of scale absmax), no systematic weight error.
Matmuls are emitted band-major over tile pairs so each weight tile is
loaded once per pair; x arrives as a host-cast fp16 tensor, two tiles per
DMA, loads on the Scalar HWDGE sequencer and stores on Sync to spread the
~600ns/DMA descriptor prep.

Self-contained: hardcodes the sharding; inputs are the full arrays as
produced by setup_inputs(); returns full (seasonal, trend) float32.
"""

import numpy as np

NCORES = 8

_prog_cache: dict = {}


# ---------------------------------------------------------------------------
# Host math
# ---------------------------------------------------------------------------

def _predict_k(x, w1, b1, w2, b2, maxK, L):
    """Per-sample kernel size, mirroring the reference MLP (float64 on host).

    round() is half-to-even in both numpy and jnp.
    """
    xg = x.astype(np.float64).mean(axis=1)
    h = np.maximum(xg @ w1.astype(np.float64) + b1.astype(np.float64), 0.0)
    z = (h @ w2.astype(np.float64) + b2.astype(np.float64))[:, 0]
    sig = 1.0 / (1.0 + np.exp(-z))
    kf = sig * (maxK - 5) + 5
    k = np.round(kf).astype(np.int64)
    k = np.clip(k, 3, min(maxK, L // 2))
    k = np.where(k % 2 == 0, k - 1, k)
    k = np.maximum(k, 3)
    return [int(v) for v in k]


def _mats_for(group, tclass, T, L):
    """Banded [128, 128] lhsT matrices for one weight group and tile class.

    group: list of (delta, weight): trend[l] += w * x[clamp(l + delta)].
    Returns {pos: [128, 128] float64} with entry [p, i] multiplying source row
    (t + pos) * 128 + p into output row t * 128 + i. Replicate-pad clamping is
    folded into the first/last tile classes.
    """
    t = {"first": 0, "mid": 1, "last": T - 1}[tclass]
    mats: dict = {}
    for i in range(128):
        l = t * 128 + i
        for d, w in group:
            g = min(max(l + d, 0), L - 1)
            rel = g - t * 128
            pos = rel // 128
            p = rel - pos * 128
            m = mats.setdefault(pos, np.zeros((128, 128), np.float64))
            m[p, i] += w
    return {pos: m for pos, m in mats.items() if np.any(m)}


def _tclass(t, T):
    return "first" if t == 0 else ("last" if t == T - 1 else "mid")


def _bf16(a):
    # 16-bit matmul-path dtype: fp16 (11-bit mantissa) — 8x lower rounding
    # noise than bf16 at identical PE/DVE throughput; all values here are
    # well inside fp16 range.
    return np.asarray(a, np.float32).astype(np.float16)


def _build_plan(x, tw, w1, b1, w2, b2):
    """All host-side math: k per sample, band matrices, row vectors, and the
    static per-tile matmul plan shared by every sample/core (union structure;
    samples lacking a slot get zero matrices).

"""
    B, L, D = x.shape
    maxK = tw.shape[1]
    assert B % NCORES == 0 and L % 128 == 0
    BPC = B // NCORES
    T = L // 128

    ks = _predict_k(x, w1, b1, w2, b2, maxK, L)
    tw64 = tw.astype(np.float64)
    E = np.exp(tw64)  # [D, maxK]

    structs = []       # per sample: {(gi, tclass, pos): mat}
    sample_resid = []  # per sample: list of residual tap columns j
    sample_invZ = []   # per sample: [D] float32
    sample_erows = []  # per sample: list of fp16-rounded E columns (float32)
    for b in range(B):
        k = ks[b]
        kh = k // 2
        const_col = [bool(np.all(tw[:, j] == tw[0, j])) for j in range(k)]
        # fp16-rounded uniform weights; Z uses the same rounded values.
        uniform = [
            (j - kh, float(_bf16(E[0, j]).astype(np.float64)))
            for j in range(k) if const_col[j]
        ]
        resid = [j for j in range(k) if not const_col[j]]
        assert max(abs(j - kh) for j in range(k)) < 128
        groups = [uniform] + [[(j - kh, 1.0)] for j in resid]
        st = {}
        for gi, grp in enumerate(groups):
            if not grp:
                continue
            for tclass in ("first", "mid", "last"):
                for pos, m in _mats_for(grp, tclass, T, L).items():
                    st[(gi, tclass, pos)] = m
        structs.append(st)
        sample_resid.append(resid)
        erows = [_bf16(E[:, j]).astype(np.float32) for j in resid]
        sample_erows.append(erows)
        Z = sum(w for _, w in uniform) + (
            np.sum([e.astype(np.float64) for e in erows], axis=0)
            if erows else 0.0)
        sample_invZ.append((1.0 / Z).astype(np.float32) * np.ones(D, np.float32))

    n_res_max = max(len(r) for r in sample_resid)
    R = 1 + n_res_max

    slot_keys = sorted(set().union(*[set(s.keys()) for s in structs]))
    slot_index = {key: i for i, key in enumerate(slot_keys)}
    n_slots = len(slot_keys)

    # Per-tile matmul plan: list over t of [(slot, gi, t_src)].
    plans = []
    for t in range(T):
        tc = _tclass(t, T)
        ops = [
            (slot_index[(gi, tcl, pos)], gi, t + pos)
            for (gi, tcl, pos) in slot_keys
            if tcl == tc
        ]
        assert ops and all(0 <= src < T for (_, _, src) in ops)
        plans.append(ops)

    # Device input arrays — everything on the matmul path is fp16. Edge-tile
    # clamp entries (m*cb) round to fp16 with <= 2.4e-4 relative error on the
    # few clamped rows; acceptable against the gate, and it keeps a single
    # dtype end-to-end.
    x16 = np.ascontiguousarray(x.astype(np.float16))
    tmats = np.zeros((B, n_slots, 128, 128), np.float16)
    rowz = np.zeros((B, 128, D), np.float16)
    rowe = np.zeros((B, max(n_res_max, 1), 128, D), np.float16)
    for b in range(B):
        for key, m in structs[b].items():
            tmats[b, slot_index[key]] = _bf16(m)
        rowz[b] = np.broadcast_to(_bf16(sample_invZ[b]), (128, D))
        for r, er in enumerate(sample_erows[b]):
            rowe[b, r] = np.broadcast_to(_bf16(er), (128, D))

    cfg = dict(
        BPC=BPC, L=L, D=D, T=T,
        n_slots=n_slots, R=R, n_res_max=n_res_max,
        plans=tuple(tuple(p) for p in plans),
    )
    return cfg, x16, tmats, rowz, rowe


# ---------------------------------------------------------------------------
# Device program
# ---------------------------------------------------------------------------

def _build_program(cfg):
    import concourse.bacc as bacc
    import concourse.mybir as mybir
    import concourse.tile as tile
    from contextlib import ExitStack

    BPC, L, D, T = cfg["BPC"], cfg["L"], cfg["D"], cfg["T"]
    n_slots, R = cfg["n_slots"], cfg["R"]
    n_res_max = cfg["n_res_max"]
    plans = cfg["plans"]
    kinds = cfg["kinds"]
    shared = cfg["shared"]
    NB = 1 if shared else BPC
    f32 = mybir.dt.float32
    f16 = mybir.dt.float16
    assert D == 512, "free width tuned for D == 512 (one PSUM bank / matmul)"

    nc = bacc.Bacc("TRN2", target_bir_lowering=False, debug=False,
                   num_devices=NCORES)
    xd = nc.dram_tensor("x16", [BPC, L, D], f16, kind="ExternalInput").ap()
    tmats = nc.dram_tensor("tmats", [NB, n_slots, 128, 128], f16,
                           kind="ExternalInput").ap()
    rowz = nc.dram_tensor("rowz", [NB, 128, D], f16,
                          kind="ExternalInput").ap()
    rowe = nc.dram_tensor("rowe", [NB, max(n_res_max, 1), 128, D], f16,
                          kind="ExternalInput").ap()
    # outs[:, 0] = trend, outs[:, 1] = seasonal — fused so each out-tile
    # stores with a single DMA instruction; fp16, upcast on the host.
    outs = nc.dram_tensor("outs", [BPC, 2, L, D], f16,
                          kind="ExternalOutput").ap()

    assert T % 2 == 0
    with tile.TileContext(nc) as tc, ExitStack() as ctx:
        tm_pool = ctx.enter_context(tc.tile_pool(name="tm", bufs=1 if shared else 3))
        row_pool = ctx.enter_context(tc.tile_pool(name="rw", bufs=1 if shared else 3))
        x_pool = ctx.enter_context(tc.tile_pool(name="xt", bufs=6))
        xs_pools = [
            ctx.enter_context(tc.tile_pool(name=f"xs{r}", bufs=7))
            for r in range(n_res_max)
        ]
        psc_pool = ctx.enter_context(tc.tile_pool(name="psc", bufs=6))
        out_pool = ctx.enter_context(tc.tile_pool(name="out", bufs=8))
        psum_pool = ctx.enter_context(
            tc.tile_pool(name="ps", bufs=8, space="PSUM"))

        nres1 = max(n_res_max, 1)
        tm = rwz = rwe = None

        def load_params(pb):
            # Loads go through the Scalar HWDGE sequencer, stores through
            # Sync — descriptor prep (~600ns/DMA) stays off the store path.
            tm_ = tm_pool.tile([128, n_slots, 128], f16, name="tm")
            nc.scalar.dma_start(tm_[:], tmats[pb].rearrange("s p i -> p s i"))
            rwz_ = row_pool.tile([128, D], f16, name="rwz", tag="rwz")
            nc.scalar.dma_start(rwz_[:], rowz[pb])
            rwe_ = row_pool.tile([128, nres1, D], f16, name="rwe", tag="rwe")
            nc.scalar.dma_start(rwe_[:], rowe[pb].rearrange("r p d -> p r d"))
            return tm_, rwz_, rwe_

        if shared:
            tm, rwz, rwe = load_params(0)

        for b in range(BPC):
            if not shared:
                tm, rwz, rwe = load_params(b)

            xpairs: dict = {}
            xst: list = [dict() for _ in range(n_res_max)]

            def get_x(t):
                # fp16 x tiles loaded two-at-a-time: one 256KB DMA per pair.
                j = t // 2
                if j not in xpairs:
                    tl = x_pool.tile([128, 2, D], f16, name="xpair")
                    nc.scalar.dma_start(
                        tl[:],
                        xd[b, j * 256:(j + 1) * 256, :]
                        .rearrange("(o p) d -> p o d", p=128))
                    xpairs[j] = tl
                return xpairs[j][:, t % 2, :]

            def get_xs(r, t):
                if t not in xst[r]:
                    tl = xs_pools[r].tile([128, D], f16, name=f"xstile{r}")
                    nc.vector.tensor_mul(tl[:], get_x(t), rwe[:, r, :])
                    xst[r][t] = tl
                return xst[r][t]

            for t in range(T):
                ps = psum_pool.tile([128, D], f32, name="ps")
                ops = plans[t]
                n_full = sum(1 for (s, _, _) in ops if kinds[s] == "full")
                for i, (slot, gi, tsrc) in enumerate(ops):
                    rhs = get_x(tsrc) if gi == 0 else get_xs(gi - 1, tsrc)[:]
                    kind = kinds[slot]
                    if kind == "full":
                        nc.tensor.matmul(
                            ps[:], tm[:, slot, :], rhs,
                            start=(i == 0), stop=(i == len(ops) - 1))
                    elif kind == "cornerA":
                        # weights rows 96-127, out rows 0-31
                        nc.tensor.matmul(
                            ps[0:32, :], tm[96:128, slot, 0:32], rhs[96:128],
                            start=False, stop=(i == len(ops) - 1),
                            tile_position=(96, 0))
                    else:  # cornerB: weights rows 0-31, out rows 96-127
                        nc.tensor.matmul(
                            ps[96:128, :], tm[0:32, slot, 96:128], rhs[0:32],
                            start=False, stop=(i == len(ops) - 1),
                            tile_position=(0, 96))
                assert n_full >= 1
                # psum -> fp16 on the otherwise-idle ScalarE, so both epilogue
                # VectorE ops run in the 2x fp16 mode.
                psc = psc_pool.tile([128, D], f16, name="psctile")
                nc.scalar.copy(psc[:], ps[:])
                duo = out_pool.tile([128, 2, D], f16, name="duo")
                nc.vector.tensor_mul(duo[:, 0, :], psc[:], rwz[:])
                nc.vector.tensor_sub(duo[:, 1, :], get_x(t), duo[:, 0, :])
                nc.sync.dma_start(
                    outs[b, :, t * 128:(t + 1) * 128, :]
                    .rearrange("o p d -> p o d"), duo[:])

                for r in range(n_res_max):
                    xst[r].pop(t - 1, None)
                if t >= 2 and t % 2 == 0:
                    xpairs.pop(t // 2 - 1, None)

    nc.compile()
    return nc


def _get_program(cfg):
    key = (cfg["BPC"], cfg["L"], cfg["D"], cfg["n_slots"],
           cfg["R"], cfg["plans"], cfg["kinds"], cfg["shared"])
    if key not in _prog_cache:
        _prog_cache[key] = _build_program(cfg)
    return _prog_cache[key]


# ---------------------------------------------------------------------------
# Entry points
# ---------------------------------------------------------------------------

def _prepare(x, trend_weights, w1, b1, w2, b2):
    x = np.ascontiguousarray(np.asarray(x, dtype=np.float32))
    tw = np.asarray(trend_weights, dtype=np.float32)
    w1 = np.asarray(w1, dtype=np.float32)
    b1 = np.asarray(b1, dtype=np.float32)
    w2 = np.asarray(w2, dtype=np.float32)
    b2 = np.asarray(b2, dtype=np.float32)

    cfg, x16, tmats, rowz, rowe = _build_plan(x, tw, w1, b1, w2, b2)
    nc = _get_program(cfg)
    BPC = cfg["BPC"]
    in_maps = []
    for c in range(NCORES):
        sl = slice(c * BPC, (c + 1) * BPC)
        pb = slice(0, 1) if cfg["shared"] else sl
        in_maps.append({
            "x16": np.ascontiguousarray(x16[sl]),
            "tmats": np.ascontiguousarray(tmats[pb]),
            "rowz": np.ascontiguousarray(rowz[pb]),
            "rowe": np.ascontiguousarray(rowe[pb]),
        })
    return nc, in_maps, cfg


def _gather(results):
    outs = np.concatenate([r["outs"] for r in results], axis=0).astype(np.float32)
    return outs[:, 1], outs[:, 0]  # (seasonal, trend)


def kernel(x, trend_weights, w1, b1, w2, b2):
    from concourse.bass_utils import run_bass_kernel_spmd

    nc, in_maps, _ = _prepare(x, trend_weights, w1, b1, w2, b2)
    res = run_bass_kernel_spmd(nc, in_maps, list(range(NCORES)))
    return _gather(res.results)


def kernel_traced(x, trend_weights, w1, b1, w2, b2, **trace_kwargs):
    """Like kernel(), but returns ((seasonal, trend), BassKernelResults) with
    an NTFF hardware profile (exec_time_ns)."""
    from concourse.bass_utils import run_bass_kernel_spmd

    nc, in_maps, _ = _prepare(x, trend_weights, w1, b1, w2, b2)
    res = run_bass_kernel_spmd(nc, in_maps, list(range(NCORES)), trace=True,
                               **trace_kwargs)
    return _gather(res.results), res


def kernel_sim(x, trend_weights, w1, b1, w2, b2, core=0):
    """CoreSim (simulator) run of a single core's program; returns that
    core's (seasonal, trend) slice."""
    from concourse.bass_interp import CoreSim

    nc, in_maps, cfg = _prepare(x, trend_weights, w1, b1, w2, b2)
    sim = CoreSim(nc, trace=False)
    for name, arr in in_maps[core].items():
        sim.tensor(name)[:] = arr
    sim.simulate(check_with_hw=False)
    outs = np.array(sim.tensor("outs")).astype(np.float32)
    return outs[:, 1], outs[:, 0]
